# revision 1
# baseline (speedup 1.0000x reference)
"""BERT self-attention (B=4, S=2048, D=1024, H=16) on 8 trn2 NeuronCores.

Sharding: core c -> (batch b = c//2, head-group hg = c%2, 8 heads each).
Each core computes out[b, :, hg*512:(hg+1)*512] independently; host
gathers. Inputs are pre-transposed on host so the contraction dim (d)
lands on SBUF partitions: xt = X.T [D,S], w{q,k,v}t = W.T shard [D,512].

On-device algorithm per core (all matmuls fp32r):
  Q^T, K^T: [o, s] pair-tiles (2 heads / 128 partitions), V: [s, o]
  augmented with a ones column per head (V_aug [s, h, 65]).
  Scores transposed per head: S^T[j, i] = K_h^T.T @ Q_h^T, head pairs
  row-packed on the two PE-array halves (K=64 each).
  U = exp(0.125*S^T + mask[j]) on ACT (mask = per-partition bias).
  ctx_u^T[dh+1, i] = V_aug.T @ U accumulated over j-tiles in PSUM; row 64
  is the softmax denominator (free rowsum via the ones column).
  Final: PE-transpose [65,128] chunks -> [128,65], DVE reciprocal of
  col 64 and tensor_scalar_mul -> out[s, o] tiles -> DMA.
"""

import numpy as np

import concourse.bass as bass
import concourse.tile as tile
from concourse import bacc, mybir
from concourse.bass_utils import run_bass_kernel_spmd
from concourse.masks import make_identity

B, S, D, H = 4, 2048, 1024, 16
DH = 64
O = 512  # per-core output width (8 heads)
HL = 8  # local heads per core
NP = 4  # head pairs per core
ST = S // 128  # 16 s-tiles
F32 = mybir.dt.float32
F32R = mybir.dt.float32r
EXP = mybir.ActivationFunctionType.Exp

_NC_CACHE = None


def build_nc():
    nc = bacc.Bacc(
        "TRN2",
        target_bir_lowering=False,
        debug=False,
        enable_asserts=True,
        num_devices=8,
    )
    xt = nc.dram_tensor("xt", [D, S], F32R, kind="ExternalInput").ap()
    wqt = nc.dram_tensor("wqt", [D, O], F32R, kind="ExternalInput").ap()
    wkt = nc.dram_tensor("wkt", [D, O], F32R, kind="ExternalInput").ap()
    wvt = nc.dram_tensor("wvt", [D, O], F32R, kind="ExternalInput").ap()
    bq = nc.dram_tensor("bq", [O], F32, kind="ExternalInput").ap()
    bk = nc.dram_tensor("bk", [O], F32, kind="ExternalInput").ap()
    bv = nc.dram_tensor("bv", [O], F32, kind="ExternalInput").ap()
    mask = nc.dram_tensor("mask", [S], F32, kind="ExternalInput").ap()
    out = nc.dram_tensor("out", [S, O], F32, kind="ExternalOutput").ap()

    with tile.TileContext(nc) as tc:
        _emit(nc, tc, xt, wqt, wkt, wvt, bq, bk, bv, mask, out)
    nc.compile()
    return nc


def _emit(nc, tc, xt, wqt, wkt, wvt, bq, bk, bv, mask, out):
    with (
        tc.tile_pool(name="singles", bufs=1) as singles,
        tc.tile_pool(name="persist", bufs=1) as persist,
        tc.tile_pool(name="psum", bufs=1, space="PSUM") as psum,
    ):
        ident = singles.tile([128, 128], F32)
        make_identity(nc, ident)
        mask_sb = singles.tile([128, ST], F32)
        nc.sync.dma_start(out=mask_sb, in_=mask.rearrange("(t p) -> p t", p=128))
        bq_sb = singles.tile([128, NP], F32)
        nc.sync.dma_start(out=bq_sb, in_=bq.rearrange("(t p) -> p t", p=128))
        bk_sb = singles.tile([128, NP], F32)
        nc.sync.dma_start(out=bk_sb, in_=bk.rearrange("(t p) -> p t", p=128))
        bv_bc = singles.tile([128, O], F32)
        nc.sync.dma_start(
            out=bv_bc, in_=bass.AP(tensor=bv.tensor, offset=0, ap=[[0, 128], [1, O]])
        )
        ones_sb = singles.tile([128, 1], F32)
        nc.vector.memset(ones_sb, 1.0)

        # persistent activations
        qts = [persist.tile([128, S], F32R, name=f"qt{p}", tag=f"qt{p}") for p in range(NP)]
        kts = [persist.tile([128, S], F32R, name=f"kt{p}", tag=f"kt{p}") for p in range(NP)]
        vaug = [
            persist.tile([128, HL, DH + 1], F32R, name=f"vaug{t}", tag=f"vaug{t}")
            for t in range(ST)
        ]

        stags = ("s0", "s1")

        with tc.tile_pool(name="proj", bufs=1) as proj:
            xts = []
            for dt in range(8):
                xti = proj.tile([128, S], F32R, name=f"xts{dt}", tag=f"xts{dt}")
                nc.sync.dma_start(out=xti, in_=xt[dt * 128 : (dt + 1) * 128, :])
                xts.append(xti)

            def load_w(wdram, label):
                wts = []
                for dt in range(8):
                    w = proj.tile([128, O], F32R, name=f"w{label}{dt}", tag="w", bufs=10)
                    nc.sync.dma_start(out=w, in_=wdram[dt * 128 : (dt + 1) * 128, :])
                    wts.append(w)
                return wts

            k = 0

            def qk_proj(wts, dsts, bias_sb, label):
                nonlocal k
                for p in range(NP):
                    for c in range(4):
                        ps = psum.tile(
                            [128, 512], F32, name=f"pp{label}{p}_{c}", tag=stags[k % 2]
                        )
                        k += 1
                        for dt in range(8):
                            nc.tensor.matmul(
                                ps,
                                wts[dt][:, p * 128 : (p + 1) * 128],
                                xts[dt][:, c * 512 : (c + 1) * 512],
                                start=(dt == 0),
                                stop=(dt == 7),
                            )
                        nc.vector.tensor_scalar_add(
                            dsts[p][:, c * 512 : (c + 1) * 512], ps, bias_sb[:, p : p + 1]
                        )

            wk_t = load_w(wkt, "k")
            qk_proj(wk_t, kts, bk_sb, "k")

            wv_t = load_w(wvt, "v")
            for st in range(ST):
                ps = psum.tile([128, O], F32, name=f"ppv{st}", tag=stags[k % 2])
                k += 1
                for dt in range(8):
                    nc.tensor.matmul(
                        ps,
                        xts[dt][:, st * 128 : (st + 1) * 128],
                        wv_t[dt],
                        start=(dt == 0),
                        stop=(dt == 7),
                    )
                va = vaug[st]
                for h in range(HL):
                    nc.vector.tensor_copy(out=va[:, h, DH : DH + 1], in_=ones_sb)
                for h in range(HL):
                    nc.vector.tensor_add(
                        va[:, h, 0:DH],
                        ps[:, h * DH : (h + 1) * DH],
                        bv_bc[:, h * DH : (h + 1) * DH],
                    )

            wq_t = load_w(wqt, "q")
            qk_proj(wq_t, qts, bq_sb, "q")

        with tc.tile_pool(name="attn", bufs=1) as attn:
            for p in range(NP):
                qtp, ktp = qts[p], kts[p]
                cxs = {
                    (ih, x): attn.tile(
                        [DH + 1, 1024], F32, name=f"cx{p}_{ih}_{x}", tag="cx", bufs=8
                    )
                    for ih in range(2)
                    for x in range(2)
                }
                for jt in range(ST):
                    for ih in range(2):
                        sps = []
                        for x in range(2):
                            sp = psum.tile(
                                [128, 1024], F32, name=f"s{p}_{ih}_{jt}_{x}",
                                tag=f"s{(2 * ih + x) % 4}"
                            )
                            sps.append(sp)
                        for c in range(2):
                            for x in range(2):
                                hp = slice(x * 64, x * 64 + 64)
                                ic = ih * 1024 + c * 512
                                nc.tensor.matmul(
                                    sps[x][:, c * 512 : (c + 1) * 512],
                                    ktp[hp, jt * 128 : (jt + 1) * 128],
                                    qtp[hp, ic : ic + 512],
                                    start=True,
                                    stop=True,
                                )
                        for x in range(2):
                            u = attn.tile(
                                [128, 1024], F32R, name=f"u{p}_{ih}_{jt}_{x}",
                                tag=f"u{x}", bufs=4
                            )
                            nc.scalar.activation(
                                u, sps[x], EXP, bias=mask_sb[:, jt : jt + 1], scale=0.125
                            )
                            pv = psum.tile(
                                [DH + 1, 1024], F32, name=f"pv{p}_{ih}_{jt}_{x}",
                                tag=f"s{(2 * ih + x) % 4}"
                            )
                            for c in range(2):
                                nc.tensor.matmul(
                                    pv[:, c * 512 : (c + 1) * 512],
                                    vaug[jt][:, 2 * p + x, :],
                                    u[:, c * 512 : (c + 1) * 512],
                                    start=True,
                                    stop=True,
                                )
                            if jt == 0:
                                nc.vector.tensor_copy(out=cxs[ih, x], in_=pv)
                            else:
                                nc.vector.tensor_add(cxs[ih, x], pv, cxs[ih, x])
                # drain: normalize + transpose + store (reads SBUF ctx directly)
                for ih in range(2):
                    for x in range(2):
                        hh = 2 * p + x
                        for it in range(8):
                            tp_ = psum.tile(
                                [128, DH + 1], F32, name=f"tp{p}_{ih}_{x}_{it}",
                                tag=f"s{it % 4}"
                            )
                            nc.tensor.transpose(
                                tp_, cxs[ih, x][:, it * 128 : (it + 1) * 128],
                                ident[0 : DH + 1, 0 : DH + 1]
                            )
                            rc = attn.tile([128, 1], F32, name=f"rc{p}_{ih}_{x}_{it}", tag="rc", bufs=6)
                            nc.vector.reciprocal(rc, tp_[:, DH : DH + 1])
                            ot = attn.tile([128, DH], F32, name=f"ot{p}_{ih}_{x}_{it}", tag="ot", bufs=6)
                            nc.vector.tensor_scalar_mul(ot, tp_[:, 0:DH], rc)
                            row = ih * 1024 + it * 128
                            nc.sync.dma_start(
                                out=out[row : row + 128, hh * DH : (hh + 1) * DH], in_=ot
                            )


def _make_in_maps(hidden_states, attention_mask, Wq, bq, Wk, bk, Wv, bv):
    in_maps = []
    for c in range(8):
        b, hg = divmod(c, 2)
        sl = slice(hg * O, (hg + 1) * O)
        in_maps.append(
            {
                "xt": np.ascontiguousarray(hidden_states[b].T),
                "wqt": np.ascontiguousarray(Wq[sl, :].T),
                "wkt": np.ascontiguousarray(Wk[sl, :].T),
                "wvt": np.ascontiguousarray(Wv[sl, :].T),
                "bq": np.ascontiguousarray(bq[sl]),
                "bk": np.ascontiguousarray(bk[sl]),
                "bv": np.ascontiguousarray(bv[sl]),
                "mask": np.ascontiguousarray(attention_mask[b, 0, 0, :]),
            }
        )
    return in_maps


def _gather(results):
    out = np.empty((B, S, D), dtype=np.float32)
    for c in range(8):
        b, hg = divmod(c, 2)
        out[b, :, hg * O : (hg + 1) * O] = results[c]["out"]
    return out


def kernel(hidden_states, attention_mask, Wq, bq, Wk, bk, Wv, bv, **run_kwargs):
    global _NC_CACHE
    args = [hidden_states, attention_mask, Wq, bq, Wk, bk, Wv, bv]
    args = [np.asarray(a, dtype=np.float32) for a in args]
    if _NC_CACHE is None:
        _NC_CACHE = build_nc()
    in_maps = _make_in_maps(*args)
    res = run_bass_kernel_spmd(_NC_CACHE, in_maps, core_ids=list(range(8)), **run_kwargs)
    kernel.last_result = res
    return _gather(res.results)



# revision 5
# speedup vs baseline: 1.6758x; 1.6758x over previous
"""BERT self-attention (B=4, S=2048, D=1024, H=16) on 8 trn2 NeuronCores.

Sharding: core c -> (batch b = c//2, head-group hg = c%2, 8 heads each).
Each core computes out[b, :, hg*512:(hg+1)*512] independently; host
gathers. Inputs are pre-transposed AND pre-cast to bf16 on host so the
contraction dim (d) lands on SBUF partitions: xt = X.T [D,S] bf16,
w{q,k,v}t = W.T shard [D,512] bf16.

On-device algorithm per core (all matmuls bf16 -> fp32 PSUM):
  Projections: Q^T, K^T [o, s] pair-tiles (2 heads / 128 partitions) in
  bf16; V as vaug [s-tile, h, 65] bf16 with a ones column per head.
  Attention per (i-chunk of 1024, head): loop over 16 j-tiles:
    scores^T[j, i] = K_h^T.T @ Q_h^T into PSUM [128, 1024],
    U = exp(0.125*S^T + mask[j]) on ACT -> bf16 SBUF (softmax numerator),
    ctx_u^T[65, i] += vaug_h.T @ U  accumulated ACROSS j-tiles in PSUM
    (start/stop flags) -- row 64 is the softmax denominator.
  The scores matmul runs one j-tile ahead of exp so ACT (the throughput
  floor: 1 elem/lane/cycle) never starves. Drain per head: DVE copy ctx
  PSUM->SBUF, PE-transpose [65,128]->[128,65], reciprocal of col 64,
  scale -> out_sb[s-tile] fp32; drain work is interleaved into the NEXT
  head's j-loop to keep both PE and ACT busy. One DMA per s-tile.
"""

from collections import deque

import numpy as np
import ml_dtypes

import concourse.bass as bass
import concourse.tile as tile
from concourse import bacc, mybir
from concourse.bass_utils import run_bass_kernel_spmd
from concourse.masks import make_identity

B, S, D, H = 4, 2048, 1024, 16
DH = 64
O = 512  # per-core output width (8 heads)
HL = 8  # local heads per core
NP = 4  # head pairs per core
ST = S // 128  # 16 s-tiles
IC = 2  # i-chunks of 1024 queries
F32 = mybir.dt.float32
BF16 = mybir.dt.bfloat16
EXP = mybir.ActivationFunctionType.Exp
BF = ml_dtypes.bfloat16

_NC_CACHE = None


def build_nc():
    nc = bacc.Bacc(
        "TRN2",
        target_bir_lowering=False,
        debug=False,
        enable_asserts=True,
        num_devices=8,
    )
    xt = nc.dram_tensor("xt", [D, S], BF16, kind="ExternalInput").ap()
    wqt = nc.dram_tensor("wqt", [D, O], BF16, kind="ExternalInput").ap()
    wkt = nc.dram_tensor("wkt", [D, O], BF16, kind="ExternalInput").ap()
    wvt = nc.dram_tensor("wvt", [D, O], BF16, kind="ExternalInput").ap()
    bq = nc.dram_tensor("bq", [O], F32, kind="ExternalInput").ap()
    bk = nc.dram_tensor("bk", [O], F32, kind="ExternalInput").ap()
    bv = nc.dram_tensor("bv", [O], F32, kind="ExternalInput").ap()
    mask = nc.dram_tensor("mask", [S], F32, kind="ExternalInput").ap()
    out = nc.dram_tensor("out", [S, O], F32, kind="ExternalOutput").ap()

    with tile.TileContext(nc) as tc:
        _emit(nc, tc, xt, wqt, wkt, wvt, bq, bk, bv, mask, out)
    nc.compile()
    return nc


def _emit(nc, tc, xt, wqt, wkt, wvt, bq, bk, bv, mask, out):
    with (
        tc.tile_pool(name="singles", bufs=1) as singles,
        tc.tile_pool(name="persist", bufs=1) as persist,
    ):
        ident = singles.tile([128, 128], F32)
        make_identity(nc, ident)
        mask_sb = singles.tile([128, ST], F32)
        nc.sync.dma_start(out=mask_sb, in_=mask.rearrange("(t p) -> p t", p=128))
        bq_sb = singles.tile([128, NP], F32)
        nc.sync.dma_start(out=bq_sb, in_=bq.rearrange("(t p) -> p t", p=128))
        bk_sb = singles.tile([128, NP], F32)
        nc.sync.dma_start(out=bk_sb, in_=bk.rearrange("(t p) -> p t", p=128))
        bv_bc = singles.tile([128, O], F32)
        nc.sync.dma_start(
            out=bv_bc, in_=bass.AP(tensor=bv.tensor, offset=0, ap=[[0, 128], [1, O]])
        )

        # persistent activations (bf16)
        qts = [persist.tile([128, S], BF16, name=f"qt{p}", tag=f"qt{p}") for p in range(NP)]
        kts = [persist.tile([128, S], BF16, name=f"kt{p}", tag=f"kt{p}") for p in range(NP)]
        vaug = [
            persist.tile([128, HL, DH + 1], BF16, name=f"vaug{t}", tag=f"vaug{t}")
            for t in range(ST)
        ]
        # final fp32 output staging, one tile per 128-row s-tile
        out_sb = [
            persist.tile([128, O], F32, name=f"osb{t}", tag=f"osb{t}") for t in range(ST)
        ]

        # ---------------- projection phase ----------------
        with (
            tc.tile_pool(name="proj", bufs=1) as proj,
            tc.tile_pool(name="ppsum", bufs=1, space="PSUM") as ppsum,
        ):
            xts = []
            for dt in range(8):
                xti = proj.tile([128, S], BF16, name=f"xts{dt}", tag=f"xts{dt}")
                nc.sync.dma_start(out=xti, in_=xt[dt * 128 : (dt + 1) * 128, :])
                xts.append(xti)
            for st in range(ST):
                nc.vector.memset(vaug[st], 1.0)

            def load_w(wdram, label):
                wts = []
                for dt in range(8):
                    w = proj.tile([128, O], BF16, name=f"w{label}{dt}", tag="w", bufs=10)
                    nc.sync.dma_start(out=w, in_=wdram[dt * 128 : (dt + 1) * 128, :])
                    wts.append(w)
                return wts

            k = 0

            def qk_proj(wts, dsts, bias_sb, label):
                nonlocal k
                for p in range(NP):
                    for c in range(4):
                        ps = ppsum.tile(
                            [128, 512], F32, name=f"pp{label}{p}_{c}", tag="pp", bufs=6
                        )
                        k += 1
                        for dt in range(8):
                            nc.tensor.matmul(
                                ps,
                                wts[dt][:, p * 128 : (p + 1) * 128],
                                xts[dt][:, c * 512 : (c + 1) * 512],
                                start=(dt == 0),
                                stop=(dt == 7),
                            )
                        nc.vector.tensor_scalar_add(
                            dsts[p][:, c * 512 : (c + 1) * 512], ps, bias_sb[:, p : p + 1]
                        )

            wk_t = load_w(wkt, "k")
            qk_proj(wk_t, kts, bk_sb, "k")

            wv_t = load_w(wvt, "v")
            for st in range(ST):
                ps = ppsum.tile([128, O], F32, name=f"ppv{st}", tag="pp", bufs=6)
                for dt in range(8):
                    nc.tensor.matmul(
                        ps,
                        xts[dt][:, st * 128 : (st + 1) * 128],
                        wv_t[dt],
                        start=(dt == 0),
                        stop=(dt == 7),
                    )
                nc.vector.tensor_add(
                    vaug[st][:, :, 0:DH],
                    ps.rearrange("p (h d) -> p h d", h=HL),
                    bv_bc.rearrange("p (h d) -> p h d", h=HL),
                )

            wq_t = load_w(wqt, "q")
            qk_proj(wq_t, qts, bq_sb, "q")

        # ---------------- attention phase ----------------
        # Per (i-chunk, head): scores^T -> exp -> pv, with exp emitted right
        # after its scores and pv LAGGING two j-tiles so it never waits on
        # the ACT->PE semaphore. ctx PSUM tiles are [128, 1024] x2 buffers;
        # after the DVE copy to SBUF the just-drained buffer doubles as the
        # transpose target, so no third PSUM pool is needed.
        with (
            tc.tile_pool(name="attn", bufs=1) as attn,
            tc.tile_pool(name="spsum", bufs=1, space="PSUM") as spsum,
            tc.tile_pool(name="cpsum", bufs=1, space="PSUM") as cpsum,
        ):
            fillers = deque()  # deferred drain ops; 1 popped per j-tile

            def run_filler():
                if fillers:
                    fillers.popleft()()

            def drain(ic, h, ctx):
                """Queue drain of head h's ctx PSUM: copy to SBUF now (DVE),
                then transpose+normalize per 128-row chunk via fillers,
                writing transposes back into the drained ctx banks."""
                cs = attn.tile(
                    [DH + 1, 1024], F32, name=f"cs{ic}_{h}", tag="cs", bufs=2
                )
                nc.vector.tensor_copy(out=cs, in_=ctx[0 : DH + 1, :])

                def tr_chunk(c4, ic=ic, h=h, cs=cs, ctx=ctx):
                    it = ic * 8 + c4
                    tp_t = ctx[:, c4 * 128 : c4 * 128 + DH + 1]
                    nc.tensor.transpose(
                        tp_t, cs[:, c4 * 128 : (c4 + 1) * 128], ident[0 : DH + 1, 0 : DH + 1]
                    )
                    rc = attn.tile([128, 1], F32, name=f"rc{ic}_{h}_{c4}", tag="rc", bufs=2)
                    nc.vector.reciprocal(rc, tp_t[:, DH : DH + 1])
                    nc.vector.tensor_scalar_mul(
                        out_sb[it][:, h * DH : (h + 1) * DH], tp_t[:, 0:DH], rc
                    )
                    if h == HL - 1:
                        nc.sync.dma_start(
                            out=out[it * 128 : (it + 1) * 128, :], in_=out_sb[it]
                        )

                for c4 in range(8):
                    fillers.append(lambda c4=c4: tr_chunk(c4))

            LAG = 2
            for ic in range(IC):
                for h in range(HL):
                    p, x = divmod(h, 2)
                    hp = slice(x * DH, x * DH + DH)
                    qtp, ktp = qts[p], kts[p]
                    ctx = cpsum.tile(
                        [128, 1024], F32, name=f"ctx{ic}_{h}", tag="ctx", bufs=2
                    )
                    us = []

                    def scores_exp(jt):
                        sp_t = spsum.tile(
                            [128, 1024], F32, name=f"s{ic}_{h}_{jt}", tag="sp", bufs=2
                        )
                        for c in range(2):
                            nc.tensor.matmul(
                                sp_t[:, c * 512 : (c + 1) * 512],
                                ktp[hp, jt * 128 : (jt + 1) * 128],
                                qtp[hp, ic * 1024 + c * 512 : ic * 1024 + (c + 1) * 512],
                                start=True,
                                stop=True,
                            )
                        u = attn.tile(
                            [128, 1024], BF16, name=f"u{ic}_{h}_{jt}", tag="u", bufs=LAG + 2
                        )
                        nc.scalar.activation(
                            u, sp_t, EXP, bias=mask_sb[:, jt : jt + 1], scale=0.125
                        )
                        us.append(u)

                    def pv(jt):
                        for c in range(2):
                            nc.tensor.matmul(
                                ctx[0 : DH + 1, c * 512 : (c + 1) * 512],
                                vaug[jt][:, h, :],
                                us[jt][:, c * 512 : (c + 1) * 512],
                                start=(jt == 0),
                                stop=(jt == ST - 1),
                            )

                    for jt in range(ST):
                        scores_exp(jt)
                        if jt >= LAG:
                            pv(jt - LAG)
                        run_filler()
                    for jt in range(ST - LAG, ST):
                        pv(jt)
                        run_filler()
                    drain(ic, h, ctx)
            while fillers:
                fillers.popleft()()


def _make_in_maps(hidden_states, attention_mask, Wq, bq, Wk, bk, Wv, bv):
    in_maps = []
    for c in range(8):
        b, hg = divmod(c, 2)
        sl = slice(hg * O, (hg + 1) * O)
        in_maps.append(
            {
                "xt": np.ascontiguousarray(hidden_states[b].T).astype(BF),
                "wqt": np.ascontiguousarray(Wq[sl, :].T).astype(BF),
                "wkt": np.ascontiguousarray(Wk[sl, :].T).astype(BF),
                "wvt": np.ascontiguousarray(Wv[sl, :].T).astype(BF),
                "bq": np.ascontiguousarray(bq[sl]),
                "bk": np.ascontiguousarray(bk[sl]),
                "bv": np.ascontiguousarray(bv[sl]),
                "mask": np.ascontiguousarray(attention_mask[b, 0, 0, :]),
            }
        )
    return in_maps


def _gather(results):
    out = np.empty((B, S, D), dtype=np.float32)
    for c in range(8):
        b, hg = divmod(c, 2)
        out[b, :, hg * O : (hg + 1) * O] = results[c]["out"]
    return out


def kernel(hidden_states, attention_mask, Wq, bq, Wk, bk, Wv, bv, **run_kwargs):
    global _NC_CACHE
    args = [hidden_states, attention_mask, Wq, bq, Wk, bk, Wv, bv]
    args = [np.asarray(a, dtype=np.float32) for a in args]
    if _NC_CACHE is None:
        _NC_CACHE = build_nc()
    in_maps = _make_in_maps(*args)
    res = run_bass_kernel_spmd(_NC_CACHE, in_maps, core_ids=list(range(8)), **run_kwargs)
    kernel.last_result = res
    return _gather(res.results)


# revision 7
# speedup vs baseline: 1.8737x; 1.1181x over previous
"""BERT self-attention (B=4, S=2048, D=1024, H=16) on 8 trn2 NeuronCores.

Sharding: core c -> (batch b = c//2, head-group hg = c%2, 8 heads each).
Each core computes out[b, :, hg*512:(hg+1)*512] independently; host
gathers. Inputs are pre-transposed AND pre-cast to bf16 on host so the
contraction dim (d) lands on SBUF partitions: xt = X.T [D,S] bf16,
w{q,k,v}t = W.T shard [D,512] bf16.

On-device algorithm per core (all matmuls bf16 -> fp32 PSUM):
  Projections: Q^T, K^T [o, s] pair-tiles (2 heads / 128 partitions) in
  bf16; V as vaug [s-tile, h, 65] bf16 with a ones column per head.
  Attention per (i-chunk of 1024, head): loop over 16 j-tiles:
    scores^T[j, i] = K_h^T.T @ Q_h^T into PSUM [128, 1024],
    U = exp(0.125*S^T + mask[j]) on ACT -> bf16 SBUF (softmax numerator),
    ctx_u^T[65, i] += vaug_h.T @ U  accumulated ACROSS j-tiles in PSUM
    (start/stop flags) -- row 64 is the softmax denominator.
  The scores matmul runs one j-tile ahead of exp so ACT (the throughput
  floor: 1 elem/lane/cycle) never starves. Drain per head: DVE copy ctx
  PSUM->SBUF, PE-transpose [65,128]->[128,65], reciprocal of col 64,
  scale -> out_sb[s-tile] fp32; drain work is interleaved into the NEXT
  head's j-loop to keep both PE and ACT busy. One DMA per s-tile.
"""

from collections import deque

import numpy as np
import ml_dtypes

import concourse.bass as bass
import concourse.tile as tile
from concourse import bacc, mybir
from concourse.bass_utils import run_bass_kernel_spmd
from concourse.masks import make_identity

B, S, D, H = 4, 2048, 1024, 16
DH = 64
O = 512  # per-core output width (8 heads)
HL = 8  # local heads per core
NP = 4  # head pairs per core
ST = S // 128  # 16 s-tiles
IC = 4  # i-chunks of 512 queries
F32 = mybir.dt.float32
BF16 = mybir.dt.bfloat16
EXP = mybir.ActivationFunctionType.Exp
BF = ml_dtypes.bfloat16

_NC_CACHE = None


def build_nc():
    nc = bacc.Bacc(
        "TRN2",
        target_bir_lowering=False,
        debug=False,
        enable_asserts=True,
        num_devices=8,
    )
    xt = nc.dram_tensor("xt", [D, S], BF16, kind="ExternalInput").ap()
    wqt = nc.dram_tensor("wqt", [D, O], BF16, kind="ExternalInput").ap()
    wkt = nc.dram_tensor("wkt", [D, O], BF16, kind="ExternalInput").ap()
    wvt = nc.dram_tensor("wvt", [D, O], BF16, kind="ExternalInput").ap()
    bq = nc.dram_tensor("bq", [O], F32, kind="ExternalInput").ap()
    bk = nc.dram_tensor("bk", [O], F32, kind="ExternalInput").ap()
    bv = nc.dram_tensor("bv", [O], F32, kind="ExternalInput").ap()
    mask = nc.dram_tensor("mask", [S], F32, kind="ExternalInput").ap()
    out = nc.dram_tensor("out", [S, O], F32, kind="ExternalOutput").ap()

    with tile.TileContext(nc) as tc:
        _emit(nc, tc, xt, wqt, wkt, wvt, bq, bk, bv, mask, out)
    nc.compile()
    return nc


def _emit(nc, tc, xt, wqt, wkt, wvt, bq, bk, bv, mask, out):
    with (
        tc.tile_pool(name="singles", bufs=1) as singles,
        tc.tile_pool(name="persist", bufs=1) as persist,
    ):
        ident = singles.tile([128, 128], F32)
        make_identity(nc, ident)
        mask_sb = singles.tile([128, ST], F32)
        nc.sync.dma_start(out=mask_sb, in_=mask.rearrange("(t p) -> p t", p=128))
        bq_sb = singles.tile([128, NP], F32)
        nc.sync.dma_start(out=bq_sb, in_=bq.rearrange("(t p) -> p t", p=128))
        bk_sb = singles.tile([128, NP], F32)
        nc.sync.dma_start(out=bk_sb, in_=bk.rearrange("(t p) -> p t", p=128))
        bv_bc = singles.tile([128, O], F32)
        nc.sync.dma_start(
            out=bv_bc, in_=bass.AP(tensor=bv.tensor, offset=0, ap=[[0, 128], [1, O]])
        )

        # persistent activations (bf16)
        qts = [persist.tile([128, S], BF16, name=f"qt{p}", tag=f"qt{p}") for p in range(NP)]
        kts = [persist.tile([128, S], BF16, name=f"kt{p}", tag=f"kt{p}") for p in range(NP)]
        vaug = [
            persist.tile([128, HL, DH + 1], BF16, name=f"vaug{t}", tag=f"vaug{t}")
            for t in range(ST)
        ]
        # final fp32 output staging, one tile per 128-row s-tile
        out_sb = [
            persist.tile([128, O], F32, name=f"osb{t}", tag=f"osb{t}") for t in range(ST)
        ]

        # ---------------- projection phase ----------------
        with (
            tc.tile_pool(name="proj", bufs=1) as proj,
            tc.tile_pool(name="ppsum", bufs=1, space="PSUM") as ppsum,
        ):
            xts = []
            for dt in range(8):
                xti = proj.tile([128, S], BF16, name=f"xts{dt}", tag=f"xts{dt}")
                nc.sync.dma_start(out=xti, in_=xt[dt * 128 : (dt + 1) * 128, :])
                xts.append(xti)
            for st in range(ST):
                nc.vector.memset(vaug[st], 1.0)

            def load_w(wdram, label):
                wts = []
                for dt in range(8):
                    w = proj.tile([128, O], BF16, name=f"w{label}{dt}", tag="w", bufs=10)
                    nc.sync.dma_start(out=w, in_=wdram[dt * 128 : (dt + 1) * 128, :])
                    wts.append(w)
                return wts

            k = 0

            def qk_proj(wts, dsts, bias_sb, label):
                nonlocal k
                for p in range(NP):
                    for c in range(4):
                        ps = ppsum.tile(
                            [128, 512], F32, name=f"pp{label}{p}_{c}", tag="pp", bufs=6
                        )
                        k += 1
                        for dt in range(8):
                            nc.tensor.matmul(
                                ps,
                                wts[dt][:, p * 128 : (p + 1) * 128],
                                xts[dt][:, c * 512 : (c + 1) * 512],
                                start=(dt == 0),
                                stop=(dt == 7),
                            )
                        nc.vector.tensor_scalar_add(
                            dsts[p][:, c * 512 : (c + 1) * 512], ps, bias_sb[:, p : p + 1]
                        )

            wk_t = load_w(wkt, "k")
            qk_proj(wk_t, kts, bk_sb, "k")

            wv_t = load_w(wvt, "v")
            for st in range(ST):
                ps = ppsum.tile([128, O], F32, name=f"ppv{st}", tag="pp", bufs=6)
                for dt in range(8):
                    nc.tensor.matmul(
                        ps,
                        xts[dt][:, st * 128 : (st + 1) * 128],
                        wv_t[dt],
                        start=(dt == 0),
                        stop=(dt == 7),
                    )
                nc.vector.tensor_add(
                    vaug[st][:, :, 0:DH],
                    ps.rearrange("p (h d) -> p h d", h=HL),
                    bv_bc.rearrange("p (h d) -> p h d", h=HL),
                )

            wq_t = load_w(wqt, "q")
            qk_proj(wq_t, qts, bq_sb, "q")

        # ---------------- attention phase ----------------
        # Heads are processed in PAIRS (h=2p on partitions 0-63, h'=2p+1 on
        # 64-127): the two K=64 scores matmuls target disjoint PE row-groups
        # (tile_position auto-derives from base_partition) and run
        # CONCURRENTLY, writing the two bank-halves of one [128, 1024] PSUM
        # tile. One exp covers both heads (mask bias per-partition = key j,
        # identical for both). pv LAGS two j-tiles behind exp so it never
        # waits on the ACT->PE semaphore; ctx accumulates in PSUM via
        # start/stop. After the drain DVE copy, the stale ctx buffer doubles
        # as the transpose target, so scores+ctx fit exactly in 8 banks.
        with (
            tc.tile_pool(name="attn", bufs=1) as attn,
            tc.tile_pool(name="spsum", bufs=1, space="PSUM") as spsum,
            tc.tile_pool(name="cpsum", bufs=1, space="PSUM") as cpsum,
        ):
            fillers = deque()  # deferred drain ops; 1 popped per j-tile

            def run_filler():
                if fillers:
                    fillers.popleft()()

            def drain(ic, h, ctx):
                """Queue drain of head h's ctx PSUM: copy to SBUF now (DVE),
                then transpose+normalize per 128-row chunk via fillers,
                writing transposes back into the drained ctx banks."""
                cs = attn.tile([DH + 1, 512], F32, name=f"cs{ic}_{h}", tag="cs", bufs=4)
                nc.vector.tensor_copy(out=cs, in_=ctx[0 : DH + 1, :])

                def tr_chunk(c4, ic=ic, h=h, cs=cs, ctx=ctx):
                    it = ic * 4 + c4
                    tp_t = ctx[:, c4 * 128 : c4 * 128 + DH + 1]
                    nc.tensor.transpose(
                        tp_t, cs[:, c4 * 128 : (c4 + 1) * 128], ident[0 : DH + 1, 0 : DH + 1]
                    )
                    rc = attn.tile([128, 1], F32, name=f"rc{ic}_{h}_{c4}", tag="rc", bufs=2)
                    nc.vector.reciprocal(rc, tp_t[:, DH : DH + 1])
                    nc.vector.tensor_scalar_mul(
                        out_sb[it][:, h * DH : (h + 1) * DH], tp_t[:, 0:DH], rc
                    )
                    if h == HL - 1:
                        nc.sync.dma_start(
                            out=out[it * 128 : (it + 1) * 128, :], in_=out_sb[it]
                        )

                for c4 in range(4):
                    fillers.append(lambda c4=c4: tr_chunk(c4))

            LAG = 2
            for ic in range(IC):
                for p in range(NP):
                    qtp, ktp = qts[p], kts[p]
                    ctxs = [
                        cpsum.tile(
                            [128, 512], F32, name=f"ctx{ic}_{p}_{x}", tag=f"cx{x}", bufs=2
                        )
                        for x in range(2)
                    ]
                    us = []

                    def scores_exp(jt):
                        sp_t = spsum.tile(
                            [128, 1024], F32, name=f"s{ic}_{p}_{jt}", tag="sp", bufs=2
                        )
                        for x in range(2):
                            hp = slice(x * DH, x * DH + DH)
                            nc.tensor.matmul(
                                sp_t[:, x * 512 : (x + 1) * 512],
                                ktp[hp, jt * 128 : (jt + 1) * 128],
                                qtp[hp, ic * 512 : (ic + 1) * 512],
                                start=True,
                                stop=True,
                            )
                        u = attn.tile(
                            [128, 1024], BF16, name=f"u{ic}_{p}_{jt}", tag="u", bufs=LAG + 2
                        )
                        nc.scalar.activation(
                            u, sp_t, EXP, bias=mask_sb[:, jt : jt + 1], scale=0.125
                        )
                        us.append(u)

                    def pv(jt):
                        for x in range(2):
                            nc.tensor.matmul(
                                ctxs[x][0 : DH + 1, :],
                                vaug[jt][:, 2 * p + x, :],
                                us[jt][:, x * 512 : (x + 1) * 512],
                                start=(jt == 0),
                                stop=(jt == ST - 1),
                            )

                    for jt in range(ST):
                        scores_exp(jt)
                        if jt >= LAG:
                            pv(jt - LAG)
                        run_filler()
                    for jt in range(ST - LAG, ST):
                        pv(jt)
                        run_filler()
                    for x in range(2):
                        drain(ic, 2 * p + x, ctxs[x])
            while fillers:
                fillers.popleft()()


def _make_in_maps(hidden_states, attention_mask, Wq, bq, Wk, bk, Wv, bv):
    in_maps = []
    for c in range(8):
        b, hg = divmod(c, 2)
        sl = slice(hg * O, (hg + 1) * O)
        in_maps.append(
            {
                "xt": np.ascontiguousarray(hidden_states[b].T).astype(BF),
                "wqt": np.ascontiguousarray(Wq[sl, :].T).astype(BF),
                "wkt": np.ascontiguousarray(Wk[sl, :].T).astype(BF),
                "wvt": np.ascontiguousarray(Wv[sl, :].T).astype(BF),
                "bq": np.ascontiguousarray(bq[sl]),
                "bk": np.ascontiguousarray(bk[sl]),
                "bv": np.ascontiguousarray(bv[sl]),
                "mask": np.ascontiguousarray(attention_mask[b, 0, 0, :]),
            }
        )
    return in_maps


def _gather(results):
    out = np.empty((B, S, D), dtype=np.float32)
    for c in range(8):
        b, hg = divmod(c, 2)
        out[b, :, hg * O : (hg + 1) * O] = results[c]["out"]
    return out


def kernel(hidden_states, attention_mask, Wq, bq, Wk, bk, Wv, bv, **run_kwargs):
    global _NC_CACHE
    args = [hidden_states, attention_mask, Wq, bq, Wk, bk, Wv, bv]
    args = [np.asarray(a, dtype=np.float32) for a in args]
    if _NC_CACHE is None:
        _NC_CACHE = build_nc()
    in_maps = _make_in_maps(*args)
    res = run_bass_kernel_spmd(_NC_CACHE, in_maps, core_ids=list(range(8)), **run_kwargs)
    kernel.last_result = res
    return _gather(res.results)
